# revision 16
# baseline (speedup 1.0000x reference)
"""Trainium2 Bass kernel for nn_HSR_2_25116968747549 (gnn_message_passing).

The reference's edge construction (`tile(B,1).reshape(2,-1)`, the preserved
index-mixing bug) makes `edge_src == edge_dst` for every edge: all edges are
self-edges, so each GATv2 layer collapses to the dense affine map
x -> (x @ Wl + bl + cb) @ linw and the whole network is

    t   = leaky_relu(x @ M1 + v1, 0.01)          M1 = Wl1@linw1@w1  (64x64)
    a   = rsqrt(mean(t^2) - mean(t)^2 + eps)     (per row; layernorm rstd)
    out = leaky_relu((a * t) @ M2c + v2, 0.01)   M2c = (I - J/64) diag(gamma) M2

Device layout (per core, 1024 rows): everything runs TRANSPOSED — features on
partitions, rows on the free dim.  The 1024 rows are stacked as two 64-
partition chunks (rows 0:512 -> partitions 0:64, rows 512:1024 -> 64:128) of
one [128, 512] tile, so each elementwise op covers the whole core's data in a
single full-width instruction and the two 64x64 matmuls per phase run
CONCURRENTLY in disjoint PE-array quadrants (tile_position packing).  Row
stats (the only partition-dim reduction) are computed with a [128,2] selector
matmul; the per-row scale is broadcast back across partitions with a [2,128]
selector matmul.  All matmul operands are bf16 (1 cycle/row vs 4 for fp32);
tolerance is 2e-2 so bf16 rounding (~0.3% norm-rel) is comfortably safe.
"""

import os

import numpy as np

B, W, D, H = 256, 32, 64, 4
N = B * W
NCORES = 8
RPC = N // NCORES          # rows per core = 1024
HALF = RPC // 2            # free dim = 512
EPS = 1e-5
SLOPE = 0.01

# Output dtype (env var for tuning only; bf16 halves the output DMA).
OUT_BF16 = os.environ.get("K_OUTBF16", "1") == "1"


def _fold_weights(inp):
    f = lambda k: np.asarray(inp[k], np.float64)
    M1 = f("Wl1") @ f("linw1") @ f("w1")
    v1 = (f("bl1") + f("cb1")) @ f("linw1") @ f("w1") + f("b1")
    A2w = f("Wl2") @ f("linw2") @ f("w2")
    M2 = f("gamma")[:, None] * A2w
    v2 = f("beta") @ A2w + (f("bl2") + f("cb2")) @ f("linw2") @ f("w2") + f("b2")
    Cm = np.eye(D) - 1.0 / D
    M2c = Cm @ M2
    return M1, v1, M2c, v2


def _edges_degenerate(src, dst):
    src = np.asarray(src)
    dst = np.asarray(dst)
    return src.shape == dst.shape and np.array_equal(src, dst) and np.all(
        np.bincount(dst.astype(np.int64), minlength=N)[:N] > 0
    )


def _numpy_fallback(inp):
    # Generic (slow) host implementation, only used if the edge arrays ever
    # stop being fully degenerate.
    x = np.asarray(inp["x"], np.float32).reshape(N, D)
    src = np.asarray(inp["edge_src"]).astype(np.int64)
    dst = np.asarray(inp["edge_dst"]).astype(np.int64)

    def gat(xf, Wl, bl, Wr, br, att, cb, linw):
        xl = (xf @ Wl + bl).reshape(N, H, D)
        xr = (xf @ Wr + br).reshape(N, H, D)
        e = xl[src] + xr[dst]
        e = np.where(e > 0, e, 0.2 * e)
        logits = np.einsum("ehd,hd->eh", e, att)
        m = np.full((N, H), -np.inf, np.float32)
        np.maximum.at(m, dst, logits)
        ex = np.exp(logits - m[dst])
        den = np.zeros((N, H), np.float32)
        np.add.at(den, dst, ex)
        alpha = ex / den[dst]
        out = np.zeros((N, H, D), np.float32)
        np.add.at(out, dst, xl[src] * alpha[:, :, None])
        return (out.reshape(N, H * D) + cb) @ linw

    g = lambda k: np.asarray(inp[k], np.float32)
    lr = lambda t, a: np.where(t > 0, t, a * t)
    out = gat(x, g("Wl1"), g("bl1"), g("Wr1"), g("br1"), g("att1"), g("cb1"), g("linw1"))
    out = lr(out @ g("w1") + g("b1"), 0.01)
    mu = out.mean(-1, keepdims=True)
    var = ((out - mu) ** 2).mean(-1, keepdims=True)
    out = (out - mu) / np.sqrt(var + EPS) * g("gamma") + g("beta")
    out = gat(out, g("Wl2"), g("bl2"), g("Wr2"), g("br2"), g("att2"), g("cb2"), g("linw2"))
    out = lr(out @ g("w2") + g("b2"), 0.01)
    return out.reshape(B, W, D).astype(np.float32)


def build_bass():
    from concourse import bacc, mybir
    import concourse.tile as tile

    fp32 = mybir.dt.float32
    bf16 = mybir.dt.bfloat16
    out_dt = bf16 if OUT_BF16 else fp32
    Act = mybir.ActivationFunctionType
    Alu = mybir.AluOpType

    nc = bacc.Bacc()
    xs_d = nc.declare_dram_parameter("xs", [128, HALF], bf16, isOutput=False)
    wp_d = nc.declare_dram_parameter("wp", [128, 258], bf16, isOutput=False)
    vp_d = nc.declare_dram_parameter("vp", [128, 4], fp32, isOutput=False)
    ys_d = nc.declare_dram_parameter("ys", [128, HALF], out_dt, isOutput=True)

    with tile.TileContext(nc) as tc:
        with (
            tc.tile_pool(name="const", bufs=1) as cpool,
            tc.tile_pool(name="psum", bufs=1, space="PSUM") as ppool,
        ):
            xsb = cpool.tile([128, HALF], bf16, tag="xsb")
            wsb = cpool.tile([128, 258], bf16, tag="wsb")
            vsb = cpool.tile([128, 4], fp32, tag="vsb")
            t_sb = cpool.tile([128, HALF], bf16, tag="t")
            sq_sb = cpool.tile([128, HALF], bf16, tag="sq")
            msq_sb = cpool.tile([2, HALF], fp32, tag="msq")
            var_sb = cpool.tile([2, HALF], fp32, tag="var")
            a_sb = cpool.tile([2, HALF], bf16, tag="a")
            u_sb = cpool.tile([128, HALF], bf16, tag="u")
            z_sb = cpool.tile([128, HALF], bf16, tag="z")
            o_sb = cpool.tile([128, HALF], out_dt, tag="o")
            warm = cpool.tile([1, 1], fp32, tag="warm")

            P1 = ppool.tile([128, HALF], fp32, tag="P1")
            Sm = ppool.tile([98, HALF], fp32, tag="Sm")
            U = ppool.tile([128, HALF], fp32, tag="U")
            Bb = ppool.tile([128, HALF], fp32, tag="Bb")

            # ---- input DMAs (SP engine), issued first so they overlap
            # the ACT table load below.
            nc.sync.dma_start(out=xsb[:], in_=xs_d[:])
            nc.sync.dma_start(out=wsb[:], in_=wp_d[:])
            nc.sync.dma_start(out=vsb[:], in_=vp_d[:])

            # ACT table warm-up: force the single table containing every
            # func we use (parametric_relu/square/abs_reciprocal_sqrt/sqrt)
            # to load while the input DMA runs.
            nc.vector.memset(warm[:], 1.0)
            nc.scalar.activation(
                out=warm[:], in_=warm[:],
                func=Act.Abs_reciprocal_sqrt,
                bias=0.0,
            )

            # Everything below is pipelined over two 256-column halves so
            # ACT/DVE/PE overlap across halves instead of serializing.
            HW2 = HALF // 2
            halves = [(0, HW2), (HW2, HALF)]

            # ---- phase 1: tT = x @ M1 (transposed); per half, two
            # concurrent 64x64-quadrant matmuls.  The sim-only wait on the
            # second half keeps prelu1_h0's semaphore wait minimal (>=2, not
            # >=4) — tile_wait_until shifts the scheduler's simulated
            # timeline so it emits tighter waits; it adds no runtime delay.
            for h, (lo, hi) in enumerate(halves):
                with tc.tile_wait_until(0.0037, enable=h == 1):
                    nc.tensor.matmul(out=P1[0:64, lo:hi],
                                     lhsT=wsb[0:64, 0:64],
                                     rhs=xsb[0:64, lo:hi],
                                     start=True, stop=True)
                    nc.tensor.matmul(out=P1[64:128, lo:hi],
                                     lhsT=wsb[64:128, 0:64],
                                     rhs=xsb[64:128, lo:hi],
                                     start=True, stop=True)

            # t = leaky_relu(tT + v1) -> bf16 ; sq = t*t
            for lo, hi in halves:
                nc.scalar.activation(
                    out=t_sb[:, lo:hi], in_=P1[:, lo:hi], func=Act.Prelu,
                    bias=vsb[:, 0:1], scale=1.0, alpha=SLOPE,
                )
                nc.vector.tensor_tensor(out=sq_sb[:, lo:hi],
                                        in0=t_sb[:, lo:hi],
                                        in1=t_sb[:, lo:hi], op=Alu.mult)

            # row means: selector matmuls (1/64 blocks -> psum receives the
            # means directly).  The four matmuls target four distinct PE
            # column-strips so they run concurrently as operands arrive.
            # S layout: means at partitions 0:2 (h0) / 64:66 (h1);
            # mean-squares at 32:34 (h0) / 96:98 (h1).
            # The sq-sums matmuls get a sim-only delay past both t-sums so
            # the msq activations' PE-counter waits stay minimal.
            for h, (lo, hi) in enumerate(halves):
                pt = 64 * h
                nc.tensor.matmul(out=Sm[pt:pt + 2, lo:hi],
                                 lhsT=wsb[:, 128:130], rhs=t_sb[:, lo:hi],
                                 start=True, stop=True,
                                 tile_position=(0, pt))
            for h, (lo, hi) in enumerate(halves):
                pt = 64 * h
                with tc.tile_wait_until(0.0054):
                    nc.tensor.matmul(out=Sm[pt + 32:pt + 34, lo:hi],
                                     lhsT=wsb[:, 128:130],
                                     rhs=sq_sb[:, lo:hi],
                                     start=True, stop=True,
                                     tile_position=(0, pt + 32))

            # u = t @ M2c (unscaled; the per-row scale commutes past M2c and
            # is applied afterwards).  tile_wait_until keeps the scheduler
            # from hoisting these ahead of the latency-critical selector
            # matmuls (u is not needed until z, ~2us later).
            with tc.tile_wait_until(0.0058):
                nc.tensor.matmul(out=U[0:64, :], lhsT=wsb[0:64, 64:128],
                                 rhs=t_sb[0:64, :], start=True, stop=True)
                nc.tensor.matmul(out=U[64:128, :], lhsT=wsb[64:128, 64:128],
                                 rhs=t_sb[64:128, :], start=True, stop=True)

            # stats: var = E[t^2] - E[t]^2 ; a = rsqrt(var + eps)
            for h, (lo, hi) in enumerate(halves):
                pt = 64 * h
                nc.scalar.activation(out=msq_sb[:, lo:hi],
                                     in_=Sm[pt:pt + 2, lo:hi],
                                     func=Act.Square, bias=vsb[0:2, 3:4])
                nc.vector.scalar_tensor_tensor(
                    out=var_sb[:, lo:hi], in0=msq_sb[:, lo:hi], scalar=-1.0,
                    in1=Sm[pt + 32:pt + 34, lo:hi],
                    op0=Alu.mult, op1=Alu.add,
                )

            # u -> sbuf bf16 (GPSIMD cannot read PSUM on trn2).  The wait
            # keeps it behind the var ops in the DVE FIFO; it then fills the
            # DVE idle window during rsqrt + broadcast, before z needs it.
            with tc.tile_wait_until(0.0066):
                nc.vector.tensor_copy(out=u_sb[:], in_=U[0:128, :])

            for lo, hi in halves:
                nc.scalar.activation(
                    out=a_sb[:, lo:hi], in_=var_sb[:, lo:hi],
                    func=Act.Abs_reciprocal_sqrt, bias=vsb[0:2, 2:3],
                )

            # broadcast a across partitions (Bb[p,j] = a[chunk(p), j]),
            # z = a*u, out = leaky_relu(z + v2)
            for lo, hi in halves:
                nc.tensor.matmul(out=Bb[0:128, lo:hi], lhsT=wsb[0:2, 130:258],
                                 rhs=a_sb[:, lo:hi], start=True, stop=True)
                nc.vector.scalar_tensor_tensor(
                    out=z_sb[:, lo:hi], in0=Bb[0:128, lo:hi], scalar=1.0,
                    in1=u_sb[:, lo:hi], op0=Alu.mult, op1=Alu.mult,
                )
                nc.scalar.activation(
                    out=o_sb[:, lo:hi], in_=z_sb[:, lo:hi], func=Act.Prelu,
                    bias=vsb[:, 1:2], scale=1.0, alpha=SLOPE,
                )

            # output DMA issued by the ACT engine itself (saves a hop).
            nc.scalar.dma_start(out=ys_d[:], in_=o_sb[:])

    return nc


def kernel(**inputs):
    if not _edges_degenerate(inputs["edge_src"], inputs["edge_dst"]):
        return _numpy_fallback(inputs)

    import ml_dtypes
    from concourse.bass_utils import run_bass_kernel_spmd

    bf16 = ml_dtypes.bfloat16
    M1, v1, M2c, v2 = _fold_weights(inputs)

    wpack = np.zeros((128, 258), np.float32)
    wpack[0:64, 0:64] = M1
    wpack[64:128, 0:64] = M1
    wpack[0:64, 64:128] = M2c
    wpack[64:128, 64:128] = M2c
    wpack[0:64, 128] = 1.0 / D          # sel2 col0: chunk0 mean
    wpack[64:128, 129] = 1.0 / D        # sel2 col1: chunk1 mean
    wpack[0, 130:130 + 64] = 1.0        # selBT row0 -> partitions 0:64
    wpack[1, 130 + 64:258] = 1.0        # selBT row1 -> partitions 64:128
    wpack = wpack.astype(bf16)

    vpack = np.zeros((128, 4), np.float32)
    vpack[0:64, 0] = v1
    vpack[64:128, 0] = v1
    vpack[0:64, 1] = v2
    vpack[64:128, 1] = v2
    vpack[:, 2] = EPS
    # col 3 stays zero (Square bias)

    xf = np.asarray(inputs["x"], np.float32).reshape(N, D)
    in_maps = []
    for c in range(NCORES):
        xc = xf[c * RPC:(c + 1) * RPC]
        xst = np.concatenate([xc[0:HALF].T, xc[HALF:].T], 0)  # [128, 512]
        in_maps.append({
            "xs": np.ascontiguousarray(xst).astype(bf16),
            "wp": wpack,
            "vp": vpack,
        })

    nc = build_bass()
    if not nc.is_finalized():
        nc.finalize()
    res = run_bass_kernel_spmd(nc, in_maps, list(range(NCORES)))
    global LAST_RESULT
    LAST_RESULT = res
    outs = []
    for r in res.results:
        ys = np.asarray(r["ys"], np.float32)          # [128, 512]
        outs.append(ys[0:64].T)                        # rows 0:512
        outs.append(ys[64:128].T)                      # rows 512:1024
    return np.concatenate(outs, 0).reshape(B, W, D).astype(np.float32)


LAST_RESULT = None


if __name__ == "__main__":
    print("kernel module ok")
